# revision 42
# baseline (speedup 1.0000x reference)
# Bloom parallel attention block on 8 trn2 NeuronCores, tensor-parallel over
# heads (2 heads per core).  Feature-major layouts, fp8 DoubleRow matmuls.
#
# All heavy matmuls run as fp8e4 (TRN FP8_EXP4, max +-240) with
# perf_mode=DoubleRow, which packs two 128-row k-tiles per instruction
# (contraction 256) and streams the moving operand at 2 values/cell:
# ~1.44x the bf16 matmul rate at FD=512.
#
# Layouts / scaling (host folds all constants):
#   - QKV weights per core are column-permuted into 6 o-tiles:
#       QA=[h0 d0:64 | h1 d0:64], QB=[h0 d64:128 | h1 d64:128], KA, KB
#       (scaled by 32*sqrt(inv_norm)), V0=h0, V1=h1 (scaled by 16).
#     This makes the PSUM evacuation of Q/K lane-aligned into the
#     [64p, 2(d-half), S] tiles that the scores DoubleRow matmul needs
#     (contraction d=128 split as two 64-partition halves), head hl at
#     base partition 64*hl.
#   - Evacuation: (psum + bias)*1/32 (q,k) or *1/16 (v) via dual-op
#     tensor_scalar -> SBUF fp8.  So SBUF holds sqrt(inv)*q, sqrt(inv)*k
#     (sigma~0.3) and v (sigma~1); scores psum = inv * q.k directly.
#   - exp on ACT in bf16 (overflow-safe); probs' = (exp(s)/32)*mask in one
#     GPSIMD scalar_tensor_tensor op -> fp8 (max score ~7 -> e^7/32=34
#     stays < 240); the 1/32 cancels between ctx and sum.  Running the
#     mask multiply on the otherwise idle GPSIMD keeps DVE headroom for
#     evacuations.
#   - alibi folds multiplicatively: V' = v*exp(alibi), ones' = exp(alibi)/16
#     => ctxn = 16*ctx (fp8-friendly range), gathered in fp8.
#   - dense: wd*16 fp8, psum = 256*out, residual pre-scaled by 256 on host,
#     final output divided by 256 on host.
#
# Pipelining identical to the bf16 version: QKV(b1) matmuls woven into
# attention(b0) k-loops, dense matmuls into attention(b1) k-loops; ctx is
# AllGathered in 6 fp8 column chunks so gathers overlap compute.
import contextlib
import os
import sys

import numpy as np

if "/opt/trn_rl_repo" not in sys.path:
    sys.path.insert(0, "/opt/trn_rl_repo")

import ml_dtypes

import concourse.bass as bass
import concourse.mybir as mybir
import concourse.tile as tile
from concourse import bacc, bass_utils

B, S, H, NH = 2, 2048, 2048, 16
HD = H // NH            # 128
NCORES = 8
HPC = NH // NCORES      # heads per core = 2
OSH = 3 * H // NCORES   # qkv output rows per core = 768
DSH = H // NCORES       # dense output cols per core = 256
P = 128
F32 = mybir.dt.float32
BF16 = mybir.dt.bfloat16
F8 = mybir.dt.float8e4
AF = mybir.ActivationFunctionType
ALU = mybir.AluOpType
DR = mybir.MatmulPerfMode.DoubleRow
NPF8 = ml_dtypes.float8_e4m3
NPBF16 = ml_dtypes.bfloat16

# which matmul sites use DoubleRow (bisection/debug knob).  scores is NOT
# DoubleRow: the 64-partition [d-half] weight layout it needs crashes the
# exec unit on hw (NRT_EXEC_UNIT_UNRECOVERABLE); plain fp8 at contraction
# 128 runs at bf16 rate, which is what the bf16 version paid anyway.
DR_SITES = set(
    s for s in os.environ.get("BASS_DR_SITES", "qkv,ctx,dense").split(",") if s
)

QK_SCALE = 32.0       # host premul on q/k weights (on top of sqrt(inv))
V_SCALE = 16.0        # host premul on v weights
WD_SCALE = 16.0       # host premul on dense weights
PROB_SCALE = 64.0     # probs' = exp(s)/PROB_SCALE (fp8 overflow guard)
OUT_SCALE = 256.0     # psum = OUT_SCALE * true output (16 ctx * 16 wd)


def build_nc():
    nc = bacc.Bacc(
        "TRN2",
        target_bir_lowering=False,
        debug=False,
        num_devices=NCORES,
    )

    hidT = nc.dram_tensor("hidT", [H, B * S], F8, kind="ExternalInput").ap()
    wqkvT = nc.dram_tensor("wqkvT", [H, OSH], F8, kind="ExternalInput").ap()
    bqkv = nc.dram_tensor("bqkv", [P, 6], F32, kind="ExternalInput").ap()
    mask01T = nc.dram_tensor("mask01T", [S, S], F8, kind="ExternalInput").ap()
    alibi_e = nc.dram_tensor("alibi_e", [P, 2 * 2 * HPC * 16], F32, kind="ExternalInput").ap()
    wdT = nc.dram_tensor("wdT", [H, DSH], F8, kind="ExternalInput").ap()
    residT = nc.dram_tensor("residT", [DSH, B * S], F32, kind="ExternalInput").ap()
    ones = nc.dram_tensor("ones", [P, P], F8, kind="ExternalInput").ap()
    eye = nc.dram_tensor("eye", [P, P], BF16, kind="ExternalInput").ap()
    outT = nc.dram_tensor("outT", [DSH, B * S], F32, kind="ExternalOutput").ap()

    with tile.TileContext(nc) as tc, contextlib.ExitStack() as es:
        ccg = [list(range(NCORES))]
        constp = es.enter_context(tc.tile_pool(name="const", bufs=1))
        dramp = es.enter_context(tc.tile_pool(name="dram", bufs=1, space="DRAM"))

        bq_sb = constp.tile([P, 6], F32)
        nc.gpsimd.dma_start(bq_sb, bqkv)
        # ale/ones/eye tiles are loaded after the first wq DMAs are queued
        # (they aren't needed until the first V evacuation, and keeping
        # them off the front of the gpsimd queue lets the first QKV
        # matmul's weights land sooner)
        ale_sb = constp.tile([P, 2 * 2 * HPC * 16], F32)
        ones_sb = constp.tile(
            [P, P], F8,
            name="ones_sb_ldw" if os.environ.get("BASS_LDW_OPT") else "ones_sb",
        )
        eye_sb = constp.tile([P, P], BF16)

        # ctx gather chunks (fp8): b0 in 2 column halves, b1 in 4 quarters
        cc_spec = [(2, S // 2), (4, S // 4)]
        cc_in = [
            [dramp.tile([HPC * HD, w], F8, name=f"cc_in{b}{i}") for i in range(n)]
            for b, (n, w) in enumerate(cc_spec)
        ]
        cc_out = [
            [
                dramp.tile([H, w], F8, addr_space="Shared", name=f"cc_out{b}{i}")
                for i in range(n)
            ]
            for b, (n, w) in enumerate(cc_spec)
        ]

        def dma_ctx(b, qc, hl, ctxn_t):
            n, w = cc_spec[b]
            chunk, qq = divmod(qc, 4 // n)
            nc.sync.dma_start(
                cc_in[b][chunk][hl * P : (hl + 1) * P, qq * 512 : (qq + 1) * 512],
                ctxn_t,
            )

        def all_gather(b, chunk):
            nc.gpsimd.collective_compute(
                "AllGather", mybir.AluOpType.bypass, replica_groups=ccg,
                ins=[cc_in[b][chunk].opt()], outs=[cc_out[b][chunk].opt()],
            )

        maskp = es.enter_context(tc.tile_pool(name="mask", bufs=1))
        qk1p = es.enter_context(tc.tile_pool(name="qk1", bufs=1))
        vtp = es.enter_context(tc.tile_pool(name="vt", bufs=1))
        v1p = es.enter_context(tc.tile_pool(name="v1", bufs=1))
        ow1p = es.enter_context(tc.tile_pool(name="ow1", bufs=1))

        mask_sb = maskp.tile([P, 16, S], F8)
        # q/k tiles: [128p(d), head, S] fp8
        q_sbs = [None, qk1p.tile([P, 2, S], F8, name="qsb1")]
        k_sbs = [None, qk1p.tile([P, 2, S], F8, name="ksb1")]
        v_sbs = [None, v1p.tile([P, HPC, 16, P], F8, name="vsb1")]
        ow_sbs = [None, ow1p.tile([P, HPC, 16, P], F8, name="owsb1")]

        def attn_block(b, qc, hl, aps, attp, extra_mm):
            """Attention for (b, head hl, q-chunk qc).  Per k-tile pair:
            2 scores DR matmuls, 1 exp (ACT bf16), 1 fused mask-scale mul
            (GPSIMD -> fp8), 1 ctx DR + 1 sum DR matmul; extra_mm(kp)
            weaves independent QKV/dense matmuls to keep the PE busy."""
            qh = q_sbs[b][:, hl, :]
            kh = k_sbs[b][:, hl, :]
            ctx_ps = aps.tile([P, 512], F32, tag="ctx", bufs=1)
            sum_ps = aps.tile([P, 512], F32, tag="sum", bufs=1)
            for kp in range(8):
                kt0 = 2 * kp
                # double-buffered scores psum: scores(kp+1) must not wait
                # for exp(kp) to drain s_ps, or the whole block pipeline
                # serializes on the ACT latency
                s_ps = aps.tile([P, 1024], F32, tag="sco", bufs=2)
                for u in range(2):
                    nc.tensor.matmul(
                        s_ps[:, u * 512 : (u + 1) * 512],
                        lhsT=kh[:, (kt0 + u) * P : (kt0 + u + 1) * P],
                        rhs=qh[:, qc * 512 : (qc + 1) * 512],
                        start=True,
                        stop=True,
                    )
                exp_t = attp.tile([P, 2, 512], BF16, tag="exp")
                nc.scalar.activation(exp_t.rearrange("p u q -> p (u q)"), s_ps, AF.Exp)
                # probs' = min(exp, 240*64) * mask/64 -> fp8 (mask01 holds
                # 1/64, exact in fp8; the min is a safety clip for the
                # score>9.6 tail that would otherwise overflow fp8 to inf)
                prob_t = attp.tile([P, 2, 512], F8, tag="prob")
                nc.vector.scalar_tensor_tensor(
                    prob_t,
                    exp_t,
                    240.0 * PROB_SCALE,
                    mask_sb[:, kt0 : kt0 + 2, qc * 512 : (qc + 1) * 512],
                    ALU.min,
                    ALU.mult,
                )
                if "ctx" in DR_SITES:
                    nc.tensor.matmul(
                        ctx_ps,
                        lhsT=v_sbs[b][:, hl, kt0 : kt0 + 2, :],
                        rhs=prob_t,
                        start=(kp == 0),
                        stop=(kp == 7),
                        perf_mode=DR,
                    )
                    nc.tensor.matmul(
                        sum_ps,
                        lhsT=ow_sbs[b][:, hl, kt0 : kt0 + 2, :],
                        rhs=prob_t,
                        start=(kp == 0),
                        stop=(kp == 7),
                        perf_mode=DR,
                    )
                else:
                    for u in range(2):
                        nc.tensor.matmul(
                            ctx_ps,
                            lhsT=v_sbs[b][:, hl, kt0 + u, :],
                            rhs=prob_t[:, u, :],
                            start=(kp == 0 and u == 0),
                            stop=(kp == 7 and u == 1),
                        )
                        nc.tensor.matmul(
                            sum_ps,
                            lhsT=ow_sbs[b][:, hl, kt0 + u, :],
                            rhs=prob_t[:, u, :],
                            start=(kp == 0 and u == 0),
                            stop=(kp == 7 and u == 1),
                        )
                extra_mm(kp)
            rec_t = attp.tile([P, 512], F32, tag="rec", bufs=2)
            nc.vector.reciprocal_approx_fast(rec_t, sum_ps)
            ctxn_t = attp.tile([P, 512], F8, tag="ctxn", bufs=2)
            nc.vector.tensor_mul(ctxn_t, ctx_ps, rec_t)
            dma_ctx(b, qc, hl, ctxn_t)

        # dense weight/residual pool opened before the phase-1 pools so the
        # pool stack unwinds LIFO (phase-1 pools close first); its DMAs are
        # issued at the start of phase 2.
        dwp = es.enter_context(tc.tile_pool(name="dw", bufs=1))

        # ---------- phase 1: QKV(b0), standalone ----------
        es1 = contextlib.ExitStack()
        qk0p = es1.enter_context(tc.tile_pool(name="qk0", bufs=1))
        v0p = es1.enter_context(tc.tile_pool(name="v0", bufs=1))
        ow0p = es1.enter_context(tc.tile_pool(name="ow0", bufs=1))
        wqp = es1.enter_context(tc.tile_pool(name="wq", bufs=1))
        hidp = es1.enter_context(tc.tile_pool(name="hid", bufs=12))
        qps = es1.enter_context(tc.tile_pool(name="qps", bufs=2, space="PSUM"))

        q_sbs[0] = qk0p.tile([P, 2, S], F8, name="qsb0")
        k_sbs[0] = qk0p.tile([P, 2, S], F8, name="ksb0")
        v_sbs[0] = v0p.tile([P, HPC, 16, P], F8, name="vsb0")
        ow_sbs[0] = ow0p.tile([P, HPC, 16, P], F8, name="owsb0")
        wq_sb = wqp.tile([P, 16, OSH], F8)

        def qkv_sc(b, sc, vT_sb):
            """QKV for one 512-wide s-chunk: 6 o-tiles x 8 h-tile pairs
            (DoubleRow); call emit(j) for j in range(48).  o-tiles 0..3 =
            QA,QB,KA,KB; 4,5 = V^T per head, which gets PE-transposed to
            V [k, d] and scaled by exp(alibi[k]); ones' built alongside."""
            hid_ts = []
            for hp in range(8):
                if b == 0 and sc == 0:
                    for ht in (2 * hp, 2 * hp + 1):
                        nc.gpsimd.dma_start(
                            wq_sb[:, ht, :], wqkvT[ht * P : (ht + 1) * P, :]
                        )
                hid_t = hidp.tile([P, 2, 512], F8, tag="hid")
                nc.sync.dma_start(
                    hid_t,
                    hidT[
                        2 * hp * P : (2 * hp + 2) * P,
                        b * S + sc * 512 : b * S + (sc + 1) * 512,
                    ].rearrange("(a p) q -> p a q", p=P),
                )
                hid_ts.append(hid_t)
            state = {"ps": None}

            def emit(j):
                ot, hp = divmod(j, 8)
                if hp == 0:
                    state["ps"] = qps.tile(
                        [P, 512], F32, tag="qkvps", bufs=2,
                        name=f"qps_{b}_{sc}_{ot}",
                    )
                if "qkv" in DR_SITES:
                    nc.tensor.matmul(
                        state["ps"],
                        lhsT=wq_sb[:, 2 * hp : 2 * hp + 2, ot * P : (ot + 1) * P],
                        rhs=hid_ts[hp],
                        start=(hp == 0),
                        stop=(hp == 7),
                        perf_mode=DR,
                    )
                else:
                    for a in range(2):
                        nc.tensor.matmul(
                            state["ps"],
                            lhsT=wq_sb[:, 2 * hp + a, ot * P : (ot + 1) * P],
                            rhs=hid_ts[hp][:, a, :],
                            start=(hp == 0 and a == 0),
                            stop=(hp == 7 and a == 1),
                        )
                if hp == 7:
                    # evacuate on DVE: (psum + bias) * (1/host scale);
                    # keeps ScalarE exclusively on Exp
                    cols = slice(sc * 512, (sc + 1) * 512)
                    if ot < 4:
                        dst = (q_sbs[b] if ot < 2 else k_sbs[b])[:, ot % 2, cols]
                        evs = 1.0 / QK_SCALE
                    else:
                        # V^T stays bf16: fp8 PE-transpose needs a stride-2
                        # output AP; the DVE scale converts to fp8 after.
                        dst = vT_sb[:, ot - 4, cols]
                        evs = 1.0 / V_SCALE
                    nc.vector.tensor_scalar(
                        dst, state["ps"], bq_sb[:, ot : ot + 1], evs,
                        ALU.add, ALU.mult,
                    )
                    if ot >= 4:
                        # V^T chunk ready: PE-transpose its 4 k-tiles (psum
                        # slots borrowed from the qkv pool), scale rows by
                        # exp(alibi); ones' = exp(alibi)/16 built alongside
                        hl = ot - 4
                        for kk in range(4):
                            kt = sc * 4 + kk
                            acol = (b * HPC + hl) * 16 + kt
                            vt_ps = qps.tile(
                                [P, P], BF16, tag="qkvps", bufs=2,
                                name=f"vt_{b}_{sc}_{hl}_{kk}",
                            )
                            nc.tensor.transpose(
                                vt_ps,
                                vT_sb[:, hl, kt * P : (kt + 1) * P],
                                eye_sb,
                            )
                            # v/ow scale on ScalarE (Copy shares the Exp
                            # act table, so no table-switch cost); frees
                            # DVE for the probs ops
                            nc.scalar.activation(
                                v_sbs[b][:, hl, kt, :],
                                vt_ps,
                                AF.Copy,
                                scale=ale_sb[:, acol : acol + 1],
                            )
                            nc.scalar.activation(
                                ow_sbs[b][:, hl, kt, :],
                                ones_sb,
                                AF.Copy,
                                scale=ale_sb[:, 64 + acol : 64 + acol + 1],
                            )

            return emit

        vT0 = vtp.tile([P, HPC, S], BF16, tag="vT", name="vT0")
        for sc in range(4):
            emit = qkv_sc(0, sc, vT0)
            if sc == 0:
                nc.gpsimd.dma_start(ale_sb, alibi_e)
                nc.gpsimd.dma_start(ones_sb, ones)
                nc.gpsimd.dma_start(eye_sb, eye)
            for j in range(48):
                emit(j)
            # mask loads on the idle gpsimd SWDGE queues, spread across
            # phase 1 so they don't delay the first attention block
            for kt in range(4 * sc, 4 * sc + 4):
                nc.gpsimd.dma_start(
                    mask_sb[:, kt, :], mask01T[kt * P : (kt + 1) * P, :]
                )

        # ---------- phase 2: attention(b0) + QKV(b1) ----------
        # dense weights/residual loaded early so phase 3 never waits
        wd_sb = dwp.tile([P, 16, DSH], F8)
        nc.sync.dma_start(wd_sb, wdT.rearrange("(ht p) o -> p ht o", p=P))
        rs_sb = dwp.tile([P, 2, B * S], F32)
        nc.sync.dma_start(rs_sb, residT.rearrange("(ot p) s -> p ot s", p=P))

        es2 = contextlib.ExitStack()
        attp = es2.enter_context(tc.tile_pool(name="att", bufs=3))
        aps = es2.enter_context(tc.tile_pool(name="aps", bufs=1, space="PSUM"))

        vT1 = vtp.tile([P, HPC, S], BF16, tag="vT", name="vT1")
        for qc in range(4):
            for hl in range(HPC):
                # 24 QKV(b1) DR matmuls woven into each block: 3 per pair
                if hl == 0:
                    emit = qkv_sc(1, qc, vT1)
                base = 24 * hl

                def extra(kp, emit=emit, base=base):
                    for j in range(3):
                        emit(base + kp * 3 + j)

                attn_block(0, qc, hl, aps, attp, extra)
            if qc == 1:
                all_gather(0, 0)

        all_gather(0, 1)
        es2.close()   # phase-2 att/aps pools
        es1.close()   # phase-1 pools (incl. qkv psum): frees PSUM for dense

        # ---------- phase 3: attention(b1) + dense(b0 + b1 early) ------
        es3 = contextlib.ExitStack()
        dctxp = es3.enter_context(tc.tile_pool(name="dctx", bufs=4))
        dps = es3.enter_context(tc.tile_pool(name="dps", bufs=2, space="PSUM"))
        doutp = es3.enter_context(tc.tile_pool(name="dout", bufs=3))
        attp = es3.enter_context(tc.tile_pool(name="att1", bufs=3))
        aps = es3.enter_context(tc.tile_pool(name="aps1", bufs=1, space="PSUM"))

        def dense_src(sc):
            if sc < 4:
                return cc_out[0][sc // 2], (sc % 2) * 512
            return cc_out[1][sc - 4], 0

        def dense_sc(sc):
            """One 512-wide output column chunk: 2 o-tiles x 8 h-tile
            pairs (DR); call emit(j) for j in range(16)."""
            src, col_off = dense_src(sc)
            state = {}

            def emit(j):
                hp, ot = divmod(j, 2)
                if ot == 0 and hp % 2 == 0:
                    # one DMA covers 4 h-tiles.  On the gpsimd SWDGE queue:
                    # these loads wait on AllGather completion, and a
                    # semaphore wait blocks the whole FIFO queue it sits on
                    # - on the sync queue it stalled every later hid/ctx/out
                    # DMA behind the gather (20us+ bubbles at phase
                    # boundaries and in the dense weave).
                    state["ctx"] = dctxp.tile(
                        [P, 4, 512], F8, tag="dctx", name="dctx_t"
                    )
                    nc.gpsimd.dma_start(
                        state["ctx"],
                        src[
                            2 * hp * P : (2 * hp + 4) * P, col_off : col_off + 512
                        ].rearrange("(a p) q -> p a q", p=P),
                    )
                if hp == 0:
                    state[f"ps{ot}"] = dps.tile(
                        [P, 512], F32, tag="dps", bufs=2,
                        name=f"dps_{sc}_{ot}",
                    )
                if "dense" in DR_SITES:
                    nc.tensor.matmul(
                        state[f"ps{ot}"],
                        lhsT=wd_sb[:, 2 * hp : 2 * hp + 2, ot * P : (ot + 1) * P],
                        rhs=state["ctx"][:, 2 * (hp % 2) : 2 * (hp % 2) + 2, :],
                        start=(hp == 0),
                        stop=(hp == 7),
                        perf_mode=DR,
                    )
                else:
                    for a in range(2):
                        nc.tensor.matmul(
                            state[f"ps{ot}"],
                            lhsT=wd_sb[:, 2 * hp + a, ot * P : (ot + 1) * P],
                            rhs=state["ctx"][:, 2 * (hp % 2) + a, :],
                            start=(hp == 0 and a == 0),
                            stop=(hp == 7 and a == 1),
                        )
                if j == 15:
                    for o in range(2):
                        o_t = doutp.tile([P, 512], F32, tag="o")
                        nc.vector.tensor_add(
                            o_t,
                            state[f"ps{o}"],
                            rs_sb[:, o, sc * 512 : (sc + 1) * 512],
                        )
                        nc.sync.dma_start(
                            outT[o * P : (o + 1) * P, sc * 512 : (sc + 1) * 512],
                            o_t,
                        )

            return emit

        # blocks 0..7 = (qc, hl); dense chunks sc0..sc1 woven into
        # blocks 2..3 (2 MMs per k-tile pair), leaving each gather time
        # to land before use.  sc2..sc6 are deliberately deferred to the
        # tail: their gathers have landed by then, and ~19us of ready PE
        # work fully covers the last gather (sc7's) latency; the unwoven
        # late blocks are ACT/DVE-bound anyway, so unweaving costs little.
        DENSE_AT = {2: 0, 3: 1}
        for qc in range(4):
            for hl in range(HPC):
                blk = qc * 2 + hl
                if blk in DENSE_AT:
                    emit = dense_sc(DENSE_AT[blk])

                    def extra(kp, emit=emit):
                        for j in range(2):
                            emit(kp * 2 + j)
                else:
                    def extra(kp):
                        pass
                attn_block(1, qc, hl, aps, attp, extra)
            # gather this q-chunk's ctx as soon as the second head is done
            all_gather(1, qc)

        # ---------- phase 4: dense tail (last b1 columns) ----------
        for sc in range(2, 8):
            emit = dense_sc(sc)
            for j in range(16):
                emit(j)

        es3.close()

    nc.compile()
    return nc


def _prep_in_maps(hidden_states, residual, alibi, attention_mask, w_qkv, b_qkv, w_dense, b_dense):
    f32 = np.float32
    hs = np.asarray(hidden_states, f32).reshape(B * S, H)
    hidT = np.ascontiguousarray(hs.T).astype(NPF8)
    mask_keep = ~np.asarray(attention_mask).reshape(S, S)
    mask01T = np.ascontiguousarray(
        mask_keep.T.astype(f32) * (1.0 / PROB_SCALE)
    ).astype(NPF8)
    ones_np = np.ones((P, P), f32).astype(NPF8)
    eye_np = np.eye(P, dtype=f32).astype(NPBF16)
    al = np.asarray(alibi, f32).reshape(B, NH, S)
    resid = np.asarray(residual, f32).reshape(B * S, H)
    wq = np.asarray(w_qkv, f32)
    bq = np.asarray(b_qkv, f32)
    wd = np.asarray(w_dense, f32)
    bd = np.asarray(b_dense, f32)
    sqinv = f32(1.0 / np.sqrt(np.sqrt(HD)))  # sqrt(inv_norm)

    in_maps = []
    for r in range(NCORES):
        wsh = wq[r * OSH : (r + 1) * OSH].reshape(HPC, 3, HD, H)
        bsh = bq[r * OSH : (r + 1) * OSH].reshape(HPC, 3, HD)
        qs = QK_SCALE * sqinv
        # o-tiles: Q(h0), Q(h1), K(h0), K(h1), V0, V1
        otiles_w = [
            wsh[0, 0] * qs,
            wsh[1, 0] * qs,
            wsh[0, 1] * qs,
            wsh[1, 1] * qs,
            wsh[0, 2] * V_SCALE,
            wsh[1, 2] * V_SCALE,
        ]
        otiles_b = [
            bsh[0, 0] * qs,
            bsh[1, 0] * qs,
            bsh[0, 1] * qs,
            bsh[1, 1] * qs,
            bsh[0, 2] * V_SCALE,
            bsh[1, 2] * V_SCALE,
        ]
        wperm = np.concatenate(otiles_w, axis=0)  # [768, H]
        bperm = np.stack(otiles_b, axis=0)        # [6, 128]
        alcols = []
        for b in range(B):
            for hl in range(HPC):
                alcols.append(np.exp(al[b, HPC * r + hl]).reshape(16, P).T)
        ale = np.concatenate(alcols, axis=1)
        ale = np.concatenate([ale, ale * (1.0 / V_SCALE)], axis=1)
        in_maps.append(
            {
                "hidT": hidT,
                "wqkvT": np.ascontiguousarray(wperm.T).astype(NPF8),
                "bqkv": np.ascontiguousarray(bperm.T),
                "mask01T": mask01T,
                "alibi_e": np.ascontiguousarray(ale),
                "wdT": np.ascontiguousarray(wd[r * DSH : (r + 1) * DSH].T * WD_SCALE).astype(NPF8),
                "residT": (
                    np.ascontiguousarray(resid[:, r * DSH : (r + 1) * DSH].T)
                    + bd[r * DSH : (r + 1) * DSH][:, None]
                ) * OUT_SCALE,
                "ones": ones_np,
                "eye": eye_np,
            }
        )
    return in_maps


if os.environ.get("BASS_LDW_OPT"):
    _orig_run_command = bass_utils.run_command

    def _run_command_ldwopt(argv, **kwargs):
        argv = [
            "--enable-ldw-opt=true" if a == "--enable-ldw-opt=false" else a
            for a in argv
        ]
        return _orig_run_command(argv, **kwargs)

    bass_utils.run_command = _run_command_ldwopt


_NC_CACHE = {}


def run(inputs: dict, trace: bool = False):
    in_maps = _prep_in_maps(**inputs)
    if "nc" not in _NC_CACHE:
        _NC_CACHE["nc"] = build_nc()
    nc = _NC_CACHE["nc"]
    res = bass_utils.run_bass_kernel_spmd(
        nc, in_maps, core_ids=list(range(NCORES)), trace=trace
    )
    out = np.empty((B * S, H), np.float32)
    for r in range(NCORES):
        out[:, r * DSH : (r + 1) * DSH] = res.results[r]["outT"].T
    out *= 1.0 / OUT_SCALE
    return out.reshape(B, S, H), res


def kernel(**inputs) -> np.ndarray:
    out, _ = run(inputs, trace=False)
    return out


# revision 43
# speedup vs baseline: 1.0213x; 1.0213x over previous
# Bloom parallel attention block on 8 trn2 NeuronCores, tensor-parallel over
# heads (2 heads per core).  Feature-major layouts, fp8 DoubleRow matmuls.
#
# All heavy matmuls run as fp8e4 (TRN FP8_EXP4, max +-240) with
# perf_mode=DoubleRow, which packs two 128-row k-tiles per instruction
# (contraction 256) and streams the moving operand at 2 values/cell:
# ~1.44x the bf16 matmul rate at FD=512.
#
# Layouts / scaling (host folds all constants):
#   - QKV weights per core are column-permuted into 6 o-tiles:
#       QA=[h0 d0:64 | h1 d0:64], QB=[h0 d64:128 | h1 d64:128], KA, KB
#       (scaled by 32*sqrt(inv_norm)), V0=h0, V1=h1 (scaled by 16).
#     This makes the PSUM evacuation of Q/K lane-aligned into the
#     [64p, 2(d-half), S] tiles that the scores DoubleRow matmul needs
#     (contraction d=128 split as two 64-partition halves), head hl at
#     base partition 64*hl.
#   - Evacuation: (psum + bias)*1/32 (q,k) or *1/16 (v) via dual-op
#     tensor_scalar -> SBUF fp8.  So SBUF holds sqrt(inv)*q, sqrt(inv)*k
#     (sigma~0.3) and v (sigma~1); scores psum = inv * q.k directly.
#   - exp on ACT in bf16 (overflow-safe); probs' = (exp(s)/32)*mask in one
#     GPSIMD scalar_tensor_tensor op -> fp8 (max score ~7 -> e^7/32=34
#     stays < 240); the 1/32 cancels between ctx and sum.  Running the
#     mask multiply on the otherwise idle GPSIMD keeps DVE headroom for
#     evacuations.
#   - alibi folds multiplicatively: V' = v*exp(alibi), ones' = exp(alibi)/16
#     => ctxn = 16*ctx (fp8-friendly range), gathered in fp8.
#   - dense: wd*16 fp8, psum = 256*out, residual pre-scaled by 256 on host,
#     final output divided by 256 on host.
#
# Pipelining identical to the bf16 version: QKV(b1) matmuls woven into
# attention(b0) k-loops, dense matmuls into attention(b1) k-loops; ctx is
# AllGathered in 6 fp8 column chunks so gathers overlap compute.
import contextlib
import os
import sys

import numpy as np

if "/opt/trn_rl_repo" not in sys.path:
    sys.path.insert(0, "/opt/trn_rl_repo")

import ml_dtypes

import concourse.bass as bass
import concourse.mybir as mybir
import concourse.tile as tile
from concourse import bacc, bass_utils

B, S, H, NH = 2, 2048, 2048, 16
HD = H // NH            # 128
NCORES = 8
HPC = NH // NCORES      # heads per core = 2
OSH = 3 * H // NCORES   # qkv output rows per core = 768
DSH = H // NCORES       # dense output cols per core = 256
P = 128
F32 = mybir.dt.float32
BF16 = mybir.dt.bfloat16
F8 = mybir.dt.float8e4
AF = mybir.ActivationFunctionType
ALU = mybir.AluOpType
DR = mybir.MatmulPerfMode.DoubleRow
NPF8 = ml_dtypes.float8_e4m3
NPBF16 = ml_dtypes.bfloat16

# which matmul sites use DoubleRow (bisection/debug knob).  scores is NOT
# DoubleRow: the 64-partition [d-half] weight layout it needs crashes the
# exec unit on hw (NRT_EXEC_UNIT_UNRECOVERABLE); plain fp8 at contraction
# 128 runs at bf16 rate, which is what the bf16 version paid anyway.
DR_SITES = set(
    s for s in os.environ.get("BASS_DR_SITES", "qkv,ctx,dense").split(",") if s
)

QK_SCALE = 32.0       # host premul on q/k weights (on top of sqrt(inv))
V_SCALE = 16.0        # host premul on v weights
WD_SCALE = 16.0       # host premul on dense weights
PROB_SCALE = 64.0     # probs' = exp(s)/PROB_SCALE (fp8 overflow guard)
OUT_SCALE = 256.0     # psum = OUT_SCALE * true output (16 ctx * 16 wd)


def build_nc():
    nc = bacc.Bacc(
        "TRN2",
        target_bir_lowering=False,
        debug=False,
        num_devices=NCORES,
    )

    hidT = nc.dram_tensor("hidT", [H, B * S], F8, kind="ExternalInput").ap()
    wqkvT = nc.dram_tensor("wqkvT", [H, OSH], F8, kind="ExternalInput").ap()
    bqkv = nc.dram_tensor("bqkv", [P, 6], F32, kind="ExternalInput").ap()
    mask01T = nc.dram_tensor("mask01T", [S, S], F8, kind="ExternalInput").ap()
    alibi_e = nc.dram_tensor("alibi_e", [P, 2 * 2 * HPC * 16], F32, kind="ExternalInput").ap()
    wdT = nc.dram_tensor("wdT", [H, DSH], F8, kind="ExternalInput").ap()
    residT = nc.dram_tensor("residT", [DSH, B * S], F32, kind="ExternalInput").ap()
    ones = nc.dram_tensor("ones", [P, P], F8, kind="ExternalInput").ap()
    eye = nc.dram_tensor("eye", [P, P], BF16, kind="ExternalInput").ap()
    outT = nc.dram_tensor("outT", [DSH, B * S], F32, kind="ExternalOutput").ap()

    with tile.TileContext(nc) as tc, contextlib.ExitStack() as es:
        ccg = [list(range(NCORES))]
        constp = es.enter_context(tc.tile_pool(name="const", bufs=1))
        dramp = es.enter_context(tc.tile_pool(name="dram", bufs=1, space="DRAM"))

        bq_sb = constp.tile([P, 6], F32)
        nc.gpsimd.dma_start(bq_sb, bqkv)
        # ale/ones/eye tiles are loaded after the first wq DMAs are queued
        # (they aren't needed until the first V evacuation, and keeping
        # them off the front of the gpsimd queue lets the first QKV
        # matmul's weights land sooner)
        ale_sb = constp.tile([P, 2 * 2 * HPC * 16], F32)
        ones_sb = constp.tile(
            [P, P], F8,
            name="ones_sb_ldw" if os.environ.get("BASS_LDW_OPT") else "ones_sb",
        )
        eye_sb = constp.tile([P, P], BF16)

        # ctx gather chunks (fp8): b0 in 2 column halves, b1 in 4 quarters
        cc_spec = [(2, S // 2), (4, S // 4)]
        cc_in = [
            [dramp.tile([HPC * HD, w], F8, name=f"cc_in{b}{i}") for i in range(n)]
            for b, (n, w) in enumerate(cc_spec)
        ]
        cc_out = [
            [
                dramp.tile([H, w], F8, addr_space="Shared", name=f"cc_out{b}{i}")
                for i in range(n)
            ]
            for b, (n, w) in enumerate(cc_spec)
        ]

        def dma_ctx(b, qc, hl, ctxn_t):
            n, w = cc_spec[b]
            chunk, qq = divmod(qc, 4 // n)
            nc.sync.dma_start(
                cc_in[b][chunk][hl * P : (hl + 1) * P, qq * 512 : (qq + 1) * 512],
                ctxn_t,
            )

        def all_gather(b, chunk):
            nc.gpsimd.collective_compute(
                "AllGather", mybir.AluOpType.bypass, replica_groups=ccg,
                ins=[cc_in[b][chunk].opt()], outs=[cc_out[b][chunk].opt()],
            )

        maskp = es.enter_context(tc.tile_pool(name="mask", bufs=1))
        qk1p = es.enter_context(tc.tile_pool(name="qk1", bufs=1))
        vtp = es.enter_context(tc.tile_pool(name="vt", bufs=1))
        v1p = es.enter_context(tc.tile_pool(name="v1", bufs=1))
        ow1p = es.enter_context(tc.tile_pool(name="ow1", bufs=1))

        mask_sb = maskp.tile([P, 16, S], F8)
        # q/k tiles: [128p(d), head, S] fp8
        q_sbs = [None, qk1p.tile([P, 2, S], F8, name="qsb1")]
        k_sbs = [None, qk1p.tile([P, 2, S], F8, name="ksb1")]
        v_sbs = [None, v1p.tile([P, HPC, 16, P], F8, name="vsb1")]
        ow_sbs = [None, ow1p.tile([P, HPC, 16, P], F8, name="owsb1")]

        def attn_block(b, qc, hl, aps, attp, extra_mm):
            """Attention for (b, head hl, q-chunk qc).  Per k-tile pair:
            2 scores DR matmuls, 1 exp (ACT bf16), 1 fused mask-scale mul
            (GPSIMD -> fp8), 1 ctx DR + 1 sum DR matmul; extra_mm(kp)
            weaves independent QKV/dense matmuls to keep the PE busy."""
            qh = q_sbs[b][:, hl, :]
            kh = k_sbs[b][:, hl, :]
            ctx_ps = aps.tile([P, 512], F32, tag="ctx", bufs=1)
            sum_ps = aps.tile([P, 512], F32, tag="sum", bufs=1)
            for kp in range(8):
                kt0 = 2 * kp
                # double-buffered scores psum: scores(kp+1) must not wait
                # for exp(kp) to drain s_ps, or the whole block pipeline
                # serializes on the ACT latency
                s_ps = aps.tile([P, 1024], F32, tag="sco", bufs=2)
                for u in range(2):
                    nc.tensor.matmul(
                        s_ps[:, u * 512 : (u + 1) * 512],
                        lhsT=kh[:, (kt0 + u) * P : (kt0 + u + 1) * P],
                        rhs=qh[:, qc * 512 : (qc + 1) * 512],
                        start=True,
                        stop=True,
                    )
                # probs' = min(exp, 240*64) * mask/64 -> fp8 (mask01 holds
                # 1/64, exact in fp8; the min is a safety clip for the
                # score>9.6 tail that would otherwise overflow fp8 to inf).
                # exp/probs run per 512-half so exp(u0) overlaps the u1
                # scores matmul and the ctx matmuls see half the stage
                # latency.
                exp_t = attp.tile([P, 2, 512], BF16, tag="exp")
                prob_t = attp.tile([P, 2, 512], F8, tag="prob")
                for u in range(2):
                    nc.scalar.activation(
                        exp_t[:, u, :], s_ps[:, u * 512 : (u + 1) * 512], AF.Exp
                    )
                    nc.vector.scalar_tensor_tensor(
                        prob_t[:, u, :],
                        exp_t[:, u, :],
                        240.0 * PROB_SCALE,
                        mask_sb[:, kt0 + u, qc * 512 : (qc + 1) * 512],
                        ALU.min,
                        ALU.mult,
                    )
                if "ctx" in DR_SITES:
                    nc.tensor.matmul(
                        ctx_ps,
                        lhsT=v_sbs[b][:, hl, kt0 : kt0 + 2, :],
                        rhs=prob_t,
                        start=(kp == 0),
                        stop=(kp == 7),
                        perf_mode=DR,
                    )
                    nc.tensor.matmul(
                        sum_ps,
                        lhsT=ow_sbs[b][:, hl, kt0 : kt0 + 2, :],
                        rhs=prob_t,
                        start=(kp == 0),
                        stop=(kp == 7),
                        perf_mode=DR,
                    )
                else:
                    for u in range(2):
                        nc.tensor.matmul(
                            ctx_ps,
                            lhsT=v_sbs[b][:, hl, kt0 + u, :],
                            rhs=prob_t[:, u, :],
                            start=(kp == 0 and u == 0),
                            stop=(kp == 7 and u == 1),
                        )
                        nc.tensor.matmul(
                            sum_ps,
                            lhsT=ow_sbs[b][:, hl, kt0 + u, :],
                            rhs=prob_t[:, u, :],
                            start=(kp == 0 and u == 0),
                            stop=(kp == 7 and u == 1),
                        )
                extra_mm(kp)
            rec_t = attp.tile([P, 512], F32, tag="rec", bufs=2)
            nc.vector.reciprocal_approx_fast(rec_t, sum_ps)
            ctxn_t = attp.tile([P, 512], F8, tag="ctxn", bufs=2)
            nc.vector.tensor_mul(ctxn_t, ctx_ps, rec_t)
            dma_ctx(b, qc, hl, ctxn_t)

        # dense weight/residual pool opened before the phase-1 pools so the
        # pool stack unwinds LIFO (phase-1 pools close first); its DMAs are
        # issued at the start of phase 2.
        dwp = es.enter_context(tc.tile_pool(name="dw", bufs=1))

        # ---------- phase 1: QKV(b0), standalone ----------
        es1 = contextlib.ExitStack()
        qk0p = es1.enter_context(tc.tile_pool(name="qk0", bufs=1))
        v0p = es1.enter_context(tc.tile_pool(name="v0", bufs=1))
        ow0p = es1.enter_context(tc.tile_pool(name="ow0", bufs=1))
        wqp = es1.enter_context(tc.tile_pool(name="wq", bufs=1))
        hidp = es1.enter_context(tc.tile_pool(name="hid", bufs=12))
        qps = es1.enter_context(tc.tile_pool(name="qps", bufs=2, space="PSUM"))

        q_sbs[0] = qk0p.tile([P, 2, S], F8, name="qsb0")
        k_sbs[0] = qk0p.tile([P, 2, S], F8, name="ksb0")
        v_sbs[0] = v0p.tile([P, HPC, 16, P], F8, name="vsb0")
        ow_sbs[0] = ow0p.tile([P, HPC, 16, P], F8, name="owsb0")
        wq_sb = wqp.tile([P, 16, OSH], F8)

        def qkv_sc(b, sc, vT_sb):
            """QKV for one 512-wide s-chunk: 6 o-tiles x 8 h-tile pairs
            (DoubleRow); call emit(j) for j in range(48).  o-tiles 0..3 =
            QA,QB,KA,KB; 4,5 = V^T per head, which gets PE-transposed to
            V [k, d] and scaled by exp(alibi[k]); ones' built alongside."""
            hid_ts = []
            for hp in range(8):
                if b == 0 and sc == 0:
                    for ht in (2 * hp, 2 * hp + 1):
                        nc.gpsimd.dma_start(
                            wq_sb[:, ht, :], wqkvT[ht * P : (ht + 1) * P, :]
                        )
                hid_t = hidp.tile([P, 2, 512], F8, tag="hid")
                nc.sync.dma_start(
                    hid_t,
                    hidT[
                        2 * hp * P : (2 * hp + 2) * P,
                        b * S + sc * 512 : b * S + (sc + 1) * 512,
                    ].rearrange("(a p) q -> p a q", p=P),
                )
                hid_ts.append(hid_t)
            state = {"ps": None}

            def emit(j):
                ot, hp = divmod(j, 8)
                if hp == 0:
                    state["ps"] = qps.tile(
                        [P, 512], F32, tag="qkvps", bufs=2,
                        name=f"qps_{b}_{sc}_{ot}",
                    )
                if "qkv" in DR_SITES:
                    nc.tensor.matmul(
                        state["ps"],
                        lhsT=wq_sb[:, 2 * hp : 2 * hp + 2, ot * P : (ot + 1) * P],
                        rhs=hid_ts[hp],
                        start=(hp == 0),
                        stop=(hp == 7),
                        perf_mode=DR,
                    )
                else:
                    for a in range(2):
                        nc.tensor.matmul(
                            state["ps"],
                            lhsT=wq_sb[:, 2 * hp + a, ot * P : (ot + 1) * P],
                            rhs=hid_ts[hp][:, a, :],
                            start=(hp == 0 and a == 0),
                            stop=(hp == 7 and a == 1),
                        )
                if hp == 7:
                    # evacuate on DVE: (psum + bias) * (1/host scale);
                    # keeps ScalarE exclusively on Exp
                    cols = slice(sc * 512, (sc + 1) * 512)
                    if ot < 4:
                        dst = (q_sbs[b] if ot < 2 else k_sbs[b])[:, ot % 2, cols]
                        evs = 1.0 / QK_SCALE
                    else:
                        # V^T stays bf16: fp8 PE-transpose needs a stride-2
                        # output AP; the DVE scale converts to fp8 after.
                        dst = vT_sb[:, ot - 4, cols]
                        evs = 1.0 / V_SCALE
                    nc.vector.tensor_scalar(
                        dst, state["ps"], bq_sb[:, ot : ot + 1], evs,
                        ALU.add, ALU.mult,
                    )
                    if ot >= 4:
                        # V^T chunk ready: PE-transpose its 4 k-tiles (psum
                        # slots borrowed from the qkv pool), scale rows by
                        # exp(alibi); ones' = exp(alibi)/16 built alongside
                        hl = ot - 4
                        for kk in range(4):
                            kt = sc * 4 + kk
                            acol = (b * HPC + hl) * 16 + kt
                            vt_ps = qps.tile(
                                [P, P], BF16, tag="qkvps", bufs=2,
                                name=f"vt_{b}_{sc}_{hl}_{kk}",
                            )
                            nc.tensor.transpose(
                                vt_ps,
                                vT_sb[:, hl, kt * P : (kt + 1) * P],
                                eye_sb,
                            )
                            # v/ow scale on ScalarE (Copy shares the Exp
                            # act table, so no table-switch cost); frees
                            # DVE for the probs ops
                            nc.scalar.activation(
                                v_sbs[b][:, hl, kt, :],
                                vt_ps,
                                AF.Copy,
                                scale=ale_sb[:, acol : acol + 1],
                            )
                            nc.scalar.activation(
                                ow_sbs[b][:, hl, kt, :],
                                ones_sb,
                                AF.Copy,
                                scale=ale_sb[:, 64 + acol : 64 + acol + 1],
                            )

            return emit

        vT0 = vtp.tile([P, HPC, S], BF16, tag="vT", name="vT0")
        for sc in range(4):
            emit = qkv_sc(0, sc, vT0)
            if sc == 0:
                nc.gpsimd.dma_start(ale_sb, alibi_e)
                nc.gpsimd.dma_start(ones_sb, ones)
                nc.gpsimd.dma_start(eye_sb, eye)
            for j in range(48):
                emit(j)
            # mask loads on the idle gpsimd SWDGE queues, spread across
            # phase 1 so they don't delay the first attention block
            for kt in range(4 * sc, 4 * sc + 4):
                nc.gpsimd.dma_start(
                    mask_sb[:, kt, :], mask01T[kt * P : (kt + 1) * P, :]
                )

        # ---------- phase 2: attention(b0) + QKV(b1) ----------
        # dense weights/residual loaded early so phase 3 never waits
        wd_sb = dwp.tile([P, 16, DSH], F8)
        nc.sync.dma_start(wd_sb, wdT.rearrange("(ht p) o -> p ht o", p=P))
        rs_sb = dwp.tile([P, 2, B * S], F32)
        nc.sync.dma_start(rs_sb, residT.rearrange("(ot p) s -> p ot s", p=P))

        es2 = contextlib.ExitStack()
        attp = es2.enter_context(tc.tile_pool(name="att", bufs=3))
        aps = es2.enter_context(tc.tile_pool(name="aps", bufs=1, space="PSUM"))

        vT1 = vtp.tile([P, HPC, S], BF16, tag="vT", name="vT1")
        for qc in range(4):
            for hl in range(HPC):
                # 24 QKV(b1) DR matmuls woven into each block: 3 per pair
                if hl == 0:
                    emit = qkv_sc(1, qc, vT1)
                base = 24 * hl

                def extra(kp, emit=emit, base=base):
                    for j in range(3):
                        emit(base + kp * 3 + j)

                attn_block(0, qc, hl, aps, attp, extra)
            if qc == 1:
                all_gather(0, 0)

        all_gather(0, 1)
        es2.close()   # phase-2 att/aps pools
        es1.close()   # phase-1 pools (incl. qkv psum): frees PSUM for dense

        # ---------- phase 3: attention(b1) + dense(b0 + b1 early) ------
        es3 = contextlib.ExitStack()
        dctxp = es3.enter_context(tc.tile_pool(name="dctx", bufs=4))
        dps = es3.enter_context(tc.tile_pool(name="dps", bufs=2, space="PSUM"))
        doutp = es3.enter_context(tc.tile_pool(name="dout", bufs=3))
        attp = es3.enter_context(tc.tile_pool(name="att1", bufs=3))
        aps = es3.enter_context(tc.tile_pool(name="aps1", bufs=1, space="PSUM"))

        def dense_src(sc):
            if sc < 4:
                return cc_out[0][sc // 2], (sc % 2) * 512
            return cc_out[1][sc - 4], 0

        def dense_sc(sc):
            """One 512-wide output column chunk: 2 o-tiles x 8 h-tile
            pairs (DR); call emit(j) for j in range(16)."""
            src, col_off = dense_src(sc)
            state = {}

            def emit(j):
                hp, ot = divmod(j, 2)
                if ot == 0 and hp % 2 == 0:
                    # one DMA covers 4 h-tiles.  On the gpsimd SWDGE queue:
                    # these loads wait on AllGather completion, and a
                    # semaphore wait blocks the whole FIFO queue it sits on
                    # - on the sync queue it stalled every later hid/ctx/out
                    # DMA behind the gather (20us+ bubbles at phase
                    # boundaries and in the dense weave).
                    state["ctx"] = dctxp.tile(
                        [P, 4, 512], F8, tag="dctx", name="dctx_t"
                    )
                    nc.gpsimd.dma_start(
                        state["ctx"],
                        src[
                            2 * hp * P : (2 * hp + 4) * P, col_off : col_off + 512
                        ].rearrange("(a p) q -> p a q", p=P),
                    )
                if hp == 0:
                    state[f"ps{ot}"] = dps.tile(
                        [P, 512], F32, tag="dps", bufs=2,
                        name=f"dps_{sc}_{ot}",
                    )
                if "dense" in DR_SITES:
                    nc.tensor.matmul(
                        state[f"ps{ot}"],
                        lhsT=wd_sb[:, 2 * hp : 2 * hp + 2, ot * P : (ot + 1) * P],
                        rhs=state["ctx"][:, 2 * (hp % 2) : 2 * (hp % 2) + 2, :],
                        start=(hp == 0),
                        stop=(hp == 7),
                        perf_mode=DR,
                    )
                else:
                    for a in range(2):
                        nc.tensor.matmul(
                            state[f"ps{ot}"],
                            lhsT=wd_sb[:, 2 * hp + a, ot * P : (ot + 1) * P],
                            rhs=state["ctx"][:, 2 * (hp % 2) + a, :],
                            start=(hp == 0 and a == 0),
                            stop=(hp == 7 and a == 1),
                        )
                if j == 15:
                    for o in range(2):
                        o_t = doutp.tile([P, 512], F32, tag="o")
                        nc.vector.tensor_add(
                            o_t,
                            state[f"ps{o}"],
                            rs_sb[:, o, sc * 512 : (sc + 1) * 512],
                        )
                        nc.sync.dma_start(
                            outT[o * P : (o + 1) * P, sc * 512 : (sc + 1) * 512],
                            o_t,
                        )

            return emit

        # blocks 0..7 = (qc, hl); dense chunks sc0..sc1 woven into
        # blocks 2..3 (2 MMs per k-tile pair), leaving each gather time
        # to land before use.  sc2..sc6 are deliberately deferred to the
        # tail: their gathers have landed by then, and ~19us of ready PE
        # work fully covers the last gather (sc7's) latency; the unwoven
        # late blocks are ACT/DVE-bound anyway, so unweaving costs little.
        DENSE_AT = {2: 0, 3: 1}
        for qc in range(4):
            for hl in range(HPC):
                blk = qc * 2 + hl
                if blk in DENSE_AT:
                    emit = dense_sc(DENSE_AT[blk])

                    def extra(kp, emit=emit):
                        for j in range(2):
                            emit(kp * 2 + j)
                else:
                    def extra(kp):
                        pass
                attn_block(1, qc, hl, aps, attp, extra)
            # gather this q-chunk's ctx as soon as the second head is done
            all_gather(1, qc)

        # ---------- phase 4: dense tail (last b1 columns) ----------
        for sc in range(2, 8):
            emit = dense_sc(sc)
            for j in range(16):
                emit(j)

        es3.close()

    nc.compile()
    return nc


def _prep_in_maps(hidden_states, residual, alibi, attention_mask, w_qkv, b_qkv, w_dense, b_dense):
    f32 = np.float32
    hs = np.asarray(hidden_states, f32).reshape(B * S, H)
    hidT = np.ascontiguousarray(hs.T).astype(NPF8)
    mask_keep = ~np.asarray(attention_mask).reshape(S, S)
    mask01T = np.ascontiguousarray(
        mask_keep.T.astype(f32) * (1.0 / PROB_SCALE)
    ).astype(NPF8)
    ones_np = np.ones((P, P), f32).astype(NPF8)
    eye_np = np.eye(P, dtype=f32).astype(NPBF16)
    al = np.asarray(alibi, f32).reshape(B, NH, S)
    resid = np.asarray(residual, f32).reshape(B * S, H)
    wq = np.asarray(w_qkv, f32)
    bq = np.asarray(b_qkv, f32)
    wd = np.asarray(w_dense, f32)
    bd = np.asarray(b_dense, f32)
    sqinv = f32(1.0 / np.sqrt(np.sqrt(HD)))  # sqrt(inv_norm)

    in_maps = []
    for r in range(NCORES):
        wsh = wq[r * OSH : (r + 1) * OSH].reshape(HPC, 3, HD, H)
        bsh = bq[r * OSH : (r + 1) * OSH].reshape(HPC, 3, HD)
        qs = QK_SCALE * sqinv
        # o-tiles: Q(h0), Q(h1), K(h0), K(h1), V0, V1
        otiles_w = [
            wsh[0, 0] * qs,
            wsh[1, 0] * qs,
            wsh[0, 1] * qs,
            wsh[1, 1] * qs,
            wsh[0, 2] * V_SCALE,
            wsh[1, 2] * V_SCALE,
        ]
        otiles_b = [
            bsh[0, 0] * qs,
            bsh[1, 0] * qs,
            bsh[0, 1] * qs,
            bsh[1, 1] * qs,
            bsh[0, 2] * V_SCALE,
            bsh[1, 2] * V_SCALE,
        ]
        wperm = np.concatenate(otiles_w, axis=0)  # [768, H]
        bperm = np.stack(otiles_b, axis=0)        # [6, 128]
        alcols = []
        for b in range(B):
            for hl in range(HPC):
                alcols.append(np.exp(al[b, HPC * r + hl]).reshape(16, P).T)
        ale = np.concatenate(alcols, axis=1)
        ale = np.concatenate([ale, ale * (1.0 / V_SCALE)], axis=1)
        in_maps.append(
            {
                "hidT": hidT,
                "wqkvT": np.ascontiguousarray(wperm.T).astype(NPF8),
                "bqkv": np.ascontiguousarray(bperm.T),
                "mask01T": mask01T,
                "alibi_e": np.ascontiguousarray(ale),
                "wdT": np.ascontiguousarray(wd[r * DSH : (r + 1) * DSH].T * WD_SCALE).astype(NPF8),
                "residT": (
                    np.ascontiguousarray(resid[:, r * DSH : (r + 1) * DSH].T)
                    + bd[r * DSH : (r + 1) * DSH][:, None]
                ) * OUT_SCALE,
                "ones": ones_np,
                "eye": eye_np,
            }
        )
    return in_maps


if os.environ.get("BASS_LDW_OPT"):
    _orig_run_command = bass_utils.run_command

    def _run_command_ldwopt(argv, **kwargs):
        argv = [
            "--enable-ldw-opt=true" if a == "--enable-ldw-opt=false" else a
            for a in argv
        ]
        return _orig_run_command(argv, **kwargs)

    bass_utils.run_command = _run_command_ldwopt


_NC_CACHE = {}


def run(inputs: dict, trace: bool = False):
    in_maps = _prep_in_maps(**inputs)
    if "nc" not in _NC_CACHE:
        _NC_CACHE["nc"] = build_nc()
    nc = _NC_CACHE["nc"]
    res = bass_utils.run_bass_kernel_spmd(
        nc, in_maps, core_ids=list(range(NCORES)), trace=trace
    )
    out = np.empty((B * S, H), np.float32)
    for r in range(NCORES):
        out[:, r * DSH : (r + 1) * DSH] = res.results[r]["outT"].T
    out *= 1.0 / OUT_SCALE
    return out.reshape(B, S, H), res


def kernel(**inputs) -> np.ndarray:
    out, _ = run(inputs, trace=False)
    return out
